# revision 8
# baseline (speedup 1.0000x reference)
"""DecoderRNN (3-step GRU decoder with Bahdanau-style attention) on 8 trn2 cores.

Sharding: pure data-parallel over batch (1024 -> 8 x 128). Weights replicated.

Host-side prep (layout only, no FLOPs): shard batch, transpose weights (the PE
contracts over the partition dim, so every `x @ W.T` needs W.T tiles with the
contraction dim on partitions), fold biases in as an extra ones-row k-tile,
gather embedding rows from `targets`, downcast to bf16/fp16, and pre-arrange
streamed weights in exact consumption order so each chunk is one large DMA.

Device-side (per core, 128 batch rows):
  prep:  encprojT[h_out, (b,s)] = w1a @ encT        (bf16 PE, fp16 resident)
  step t:
    hproj = h @ w1b.T + b1 (PE, b-part) -> transpose -> hprojT
    X = encprojT + hprojT (bcast over s); x = tanh(X)     (DVE + ACT)
    e = w2^T x (PE, M=1) -> DRAM bounce [1,BS] -> [128,S] -> softmax
    ctx = sum_s diag(alpha_s) @ enc_s   (PE, PSUM accum over s)
    GRU gates + h update (PE b-part, sigmoid via tanh identity)
    logits = h_new @ outW.T + b (PE); keep max/sumexp; raw logits -> DRAM
  tail: logp = logits - max - ln(sumexp)   (single ACT Ln table switch)
"""

import os
import numpy as np
import ml_dtypes

import concourse.bass as bass
import concourse.tile as tile
from concourse import bacc, mybir
from concourse.masks import make_identity

F32 = mybir.dt.float32
F16 = mybir.dt.float16
BF16 = mybir.dt.bfloat16
AF = mybir.ActivationFunctionType
ALU = mybir.AluOpType
AX = mybir.AxisListType

NCORES = 8
B, S, H, DW, V = 1024, 40, 1024, 512, 4000
BL = B // NCORES          # 128 batch rows per core
P = 128                   # partitions
KH = H // P               # 8 k-tiles over H
BS = BL * S               # 5120
MAXLEN = 3
KX = DW // P + KH + 1     # 13 k-tiles for [emb; ctx; bias]
KHH = KH + 1              # 9 k-tiles for [h; bias]
G3 = 3 * H                # 3072
VP = 4096                 # padded vocab (8 x 512)
NVC = VP // 512
# score chunks over BS, 40-aligned so the hproj broadcast is whole-b
SCORE_CHUNKS = [(i * 480, 480) for i in range(10)] + [(4800, 320)]
WSLOT = 13 * 512          # streamed-weight SBUF slot (columns)

bf = ml_dtypes.bfloat16


def _tiled(mat, ktiles, width):
    """[ktiles*P, width] -> [P, ktiles*width], k-tile k at cols [k*w,(k+1)*w)."""
    assert mat.shape == (ktiles * P, width)
    return np.ascontiguousarray(
        mat.reshape(ktiles, P, width).transpose(1, 0, 2).reshape(P, ktiles * width)
    )


def _aug(wT, bias, ktiles):
    """Stack [wT; bias row; zero pad] to ktiles*P rows."""
    rows = wT.shape[0] + 1
    pad = ktiles * P - rows
    return np.vstack([wT, bias[None, :], np.zeros((pad, wT.shape[1]), wT.dtype)])


def _chunk_stream(aug, ktiles, chunks):
    """[ktiles*P, W] -> [P, len(chunks)*ktiles*512]: per chunk, all k slices."""
    a = aug.reshape(ktiles, P, aug.shape[1])
    blocks = []
    for c in chunks:
        for k in range(ktiles):
            blocks.append(a[k][:, c * 512:(c + 1) * 512])
    return np.concatenate(blocks, axis=1)


def _gru_stream_layout(wihT_aug, whhT_aug):
    """Device consumption order: rz c=0..3: [wih k 0..12][whh k 0..8];
    then i_n c=4,5 (wih only); then h_n c=4,5 (whh only). [P, 132*512]."""
    wih = wihT_aug.reshape(KX, P, G3)
    whh = whhT_aug.reshape(KHH, P, G3)
    blocks = []
    for c in range(4):
        for k in range(KX):
            blocks.append(wih[k][:, c * 512:(c + 1) * 512])
        for k in range(KHH):
            blocks.append(whh[k][:, c * 512:(c + 1) * 512])
    for c in range(4, 6):
        for k in range(KX):
            blocks.append(wih[k][:, c * 512:(c + 1) * 512])
    for c in range(4, 6):
        for k in range(KHH):
            blocks.append(whh[k][:, c * 512:(c + 1) * 512])
    return np.concatenate(blocks, axis=1)


def _enc_T_layout(enc_c):
    """enc [BL,S,H] -> encT stream [P, 10*KH*512]: n-chunk-major, then k."""
    encT = enc_c.transpose(2, 0, 1).reshape(H, BS)          # [h, b*40+s]
    t = encT.reshape(KH, P, BS)
    blocks = []
    for n in range(BS // 512):
        for k in range(KH):
            blocks.append(t[k][:, n * 512:(n + 1) * 512])
    return np.concatenate(blocks, axis=1)


def prep_core_inputs(inputs, c):
    f32 = np.float32
    enc = np.asarray(inputs["encoder_outputs"], f32)[c * BL:(c + 1) * BL]
    h0 = np.asarray(inputs["encoder_hidden"], f32)[0, c * BL:(c + 1) * BL]
    targets = np.asarray(inputs["targets"]).astype(np.int64)[:, c * BL:(c + 1) * BL]
    emb = np.asarray(inputs["emb"], f32)
    w1 = np.asarray(inputs["attn_w1"], f32)
    b1 = np.asarray(inputs["attn_b1"], f32)
    w2 = np.asarray(inputs["attn_w2"], f32)

    d = {}
    d["encT"] = np.ascontiguousarray(_enc_T_layout(enc)).astype(bf)
    d["encN"] = np.ascontiguousarray(enc.reshape(BL, S * H)).astype(bf)
    d["w1aT"] = _tiled(np.ascontiguousarray(w1[:, :H].T), KH, H).astype(bf)
    w1bT_aug = _aug(np.ascontiguousarray(w1[:, H:].T), b1, KHH)
    d["w1b"] = np.ascontiguousarray(
        _chunk_stream(w1bT_aug, KHH, range(H // 512))).astype(bf)
    d["w2T"] = np.ascontiguousarray(w2[0].reshape(KH, P).T).astype(np.float16)
    d["h0T"] = _tiled(np.ascontiguousarray(h0.T), KH, BL).astype(bf)
    d["h0N"] = np.ascontiguousarray(h0)  # f32
    embT_steps = []
    for t in range(MAXLEN):
        if t == 0:
            rows = emb[np.zeros(BL, np.int64)]
        elif t == 1:
            rows = emb[targets[0] + 1]
        else:
            rows = emb[targets[1] + 36]
        embT_steps.append(_tiled(np.ascontiguousarray(rows.T), DW // P, BL))
    d["embT"] = np.concatenate(embT_steps, axis=1).astype(bf)
    wihT_aug = _aug(np.ascontiguousarray(np.asarray(inputs["w_ih"], f32).T),
                    np.asarray(inputs["b_ih"], f32), KX)
    whhT_aug = _aug(np.ascontiguousarray(np.asarray(inputs["w_hh"], f32).T),
                    np.asarray(inputs["b_hh"], f32), KHH)
    d["wg"] = np.ascontiguousarray(_gru_stream_layout(wihT_aug, whhT_aug)).astype(bf)
    for name, wk, bk in (("o0T", "out0_w", "out0_b"), ("o1T", "out1_w", "out1_b")):
        oT = _aug(np.ascontiguousarray(np.asarray(inputs[wk], f32).T),
                  np.asarray(inputs[bk], f32), KHH)          # [1152, 4000]
        oTp = np.zeros((KHH * P, VP), np.float32)
        oTp[:, :V] = oT
        d[name] = np.ascontiguousarray(
            _chunk_stream(oTp, KHH, range(NVC))).astype(bf)
    return d


def build_nc():
    nc = bacc.Bacc("TRN2", target_bir_lowering=False, debug=False)
    d = {}
    d["encT"] = nc.dram_tensor("encT", [P, (BS // 512) * KH * 512], BF16, kind="ExternalInput").ap()
    d["encN"] = nc.dram_tensor("encN", [P, S * H], BF16, kind="ExternalInput").ap()
    d["w1aT"] = nc.dram_tensor("w1aT", [P, KH * H], BF16, kind="ExternalInput").ap()
    d["w1b"] = nc.dram_tensor("w1b", [P, 2 * KHH * 512], BF16, kind="ExternalInput").ap()
    d["w2T"] = nc.dram_tensor("w2T", [P, KH], F16, kind="ExternalInput").ap()
    d["h0T"] = nc.dram_tensor("h0T", [P, KH * P], BF16, kind="ExternalInput").ap()
    d["h0N"] = nc.dram_tensor("h0N", [P, H], F32, kind="ExternalInput").ap()
    d["embT"] = nc.dram_tensor("embT", [P, MAXLEN * (DW // P) * P], BF16, kind="ExternalInput").ap()
    d["wg"] = nc.dram_tensor("wg", [P, 132 * 512], BF16, kind="ExternalInput").ap()
    d["o0T"] = nc.dram_tensor("o0T", [P, NVC * KHH * 512], BF16, kind="ExternalInput").ap()
    d["o1T"] = nc.dram_tensor("o1T", [P, NVC * KHH * 512], BF16, kind="ExternalInput").ap()
    d["out"] = nc.dram_tensor("out", [MAXLEN, BL, V], F32, kind="ExternalOutput").ap()
    d["e"] = nc.dram_tensor("e_scr", [1, BS], F16, kind="Internal").ap()
    d["lo"] = nc.dram_tensor("lo_scr", [MAXLEN, BL, VP], F16, kind="Internal").ap()

    with tile.TileContext(nc) as tc:
        _emit(tc, d)
    nc.compile()
    return nc


def _bcast_s(ap2d):
    """[P, nb] AP -> [P, nb, S] AP broadcasting over a new trailing s dim."""
    return bass.AP(ap2d.tensor, ap2d.offset,
                   [list(ap2d.ap[0]), list(ap2d.ap[1]), [0, S]])


def _emit(tc, d):
    nc = tc.nc
    from contextlib import ExitStack
    ctx = ExitStack()
    with ctx:
        res = ctx.enter_context(tc.tile_pool(name="res", bufs=1))
        scr = ctx.enter_context(tc.tile_pool(name="scr", bufs=1))
        tiny = ctx.enter_context(tc.tile_pool(name="tiny", bufs=2))
        pp_mm = ctx.enter_context(tc.tile_pool(name="pp_mm", bufs=3, space="PSUM"))
        pp_ctx = ctx.enter_context(tc.tile_pool(name="pp_ctx", bufs=2, space="PSUM"))
        pp_e = ctx.enter_context(tc.tile_pool(name="pp_e", bufs=2, space="PSUM"))
        pp_tr = ctx.enter_context(tc.tile_pool(name="pp_tr", bufs=1, space="PSUM"))
        xpool = ctx.enter_context(tc.tile_pool(name="xpool", bufs=4))

        # ---- static/resident tiles ----
        I_bf = res.tile([P, P], BF16, tag="I_bf")
        make_identity(nc, I_bf[:])
        I_f16 = res.tile([P, P], F16, tag="I_f16")
        make_identity(nc, I_f16[:])
        ones_sb = res.tile([P, P], BF16, tag="ones")
        nc.gpsimd.memset(ones_sb[:], 0.0)
        nc.gpsimd.memset(ones_sb[0:1, :], 1.0)

        encprojT = res.tile([P, KH * BS], F16, tag="encprojT")   # 80KB/part
        hprojT = res.tile([P, H], F16, tag="hprojT")
        hT_sb = res.tile([P, KH * P], BF16, tag="hT")
        hN_sb = res.tile([P, H], F32, tag="hN")
        embT_sb = res.tile([P, MAXLEN * (DW // P) * P], BF16, tag="embT")
        ctxT_sb = res.tile([P, H], BF16, tag="ctxT")
        w2T_sb = res.tile([P, KH], F16, tag="w2T")
        e_sb = res.tile([1, BS], F16, tag="e_sb")
        alpha_sb = res.tile([P, S], F32, tag="alpha")
        lo_sb = res.tile([P, VP], F16, tag="lo_sb")
        rz_sb = res.tile([P, 2 * H], F16, tag="rz")
        in_sb = res.tile([P, H], F16, tag="in_sb")
        stats = [
            (res.tile([P, 1], F32, tag=f"negmax{t}", name=f"negmax{t}"),
             res.tile([P, 1], F32, tag=f"se{t}", name=f"se{t}"))
            for t in range(MAXLEN)
        ]

        nc.sync.dma_start(out=hT_sb[:], in_=d["h0T"])
        nc.sync.dma_start(out=hN_sb[:], in_=d["h0N"])
        nc.sync.dma_start(out=embT_sb[:], in_=d["embT"])
        nc.sync.dma_start(out=w2T_sb[:], in_=d["w2T"])

        # ---- prep: encprojT = w1a @ encT ----
        # prep-only pool is scoped so wbuf/encbuf (created after) reuse its SBUF
        with tc.tile_pool(name="prep", bufs=2) as prep:
            w1aT_sb = prep.tile([P, KH * H], BF16, tag="w1aT", bufs=1)
            nc.sync.dma_start(out=w1aT_sb[:], in_=d["w1aT"])
            for n in range(BS // 512):
                eT = prep.tile([P, KH * 512], BF16, tag="encTn")
                nc.sync.dma_start(out=eT[:], in_=d["encT"][:, n * KH * 512:(n + 1) * KH * 512])
                for m in range(KH):
                    ps = pp_mm.tile([P, 512], F32, tag="ps")
                    for k in range(KH):
                        nc.tensor.matmul(
                            ps[:],
                            w1aT_sb[:, k * H + m * P: k * H + (m + 1) * P],
                            eT[:, k * 512:(k + 1) * 512],
                            start=(k == 0), stop=(k == KH - 1),
                        )
                    dst = encprojT[:, m * BS + n * 512: m * BS + (n + 1) * 512]
                    if (n * KH + m) % 2 == 0:
                        nc.scalar.copy(dst, ps[:])
                    else:
                        nc.vector.tensor_copy(dst, ps[:])

        wbuf = ctx.enter_context(tc.tile_pool(name="wbuf", bufs=2))
        encbuf = ctx.enter_context(tc.tile_pool(name="encbuf", bufs=2))

        # ---- decode steps ----
        for t in range(MAXLEN):
            gh_lhs = [hT_sb[:, j * P:(j + 1) * P] for j in range(KH)] + [ones_sb[:]]

            # hproj (b-part): [128b, 1024] = h @ w1b.T + b1, then transpose
            hpN = scr.tile([P, H], F16, tag="hpN")
            for c in range(2):
                wt = wbuf.tile([P, WSLOT], BF16, tag="wbuf")
                nc.sync.dma_start(out=wt[:, :KHH * 512],
                                  in_=d["w1b"][:, c * KHH * 512:(c + 1) * KHH * 512])
                ps = pp_mm.tile([P, 512], F32, tag="ps")
                for k in range(KHH):
                    nc.tensor.matmul(ps[:], gh_lhs[k], wt[:, k * 512:(k + 1) * 512],
                                     start=(k == 0), stop=(k == KHH - 1))
                nc.vector.tensor_copy(hpN[:, c * 512:(c + 1) * 512], ps[:])
            for j in range(KH):
                tp = pp_tr.tile([P, P], F16, tag="tp")
                nc.tensor.transpose(tp[:], hpN[:, j * P:(j + 1) * P], I_f16[:])
                nc.scalar.copy(hprojT[:, j * P:(j + 1) * P], tp[:])

            # attention scores: e = w2^T tanh(encprojT + hprojT)
            for (c0, cn) in SCORE_CHUNKS:
                nb = cn // S
                b0 = c0 // S
                pe = pp_e.tile([1, 512], F32, tag="pe")
                for m in range(KH):
                    X = xpool.tile([P, 480], F16, tag="X")
                    src = encprojT[:, m * BS + c0: m * BS + c0 + cn]
                    hp = _bcast_s(hprojT[:, m * P + b0: m * P + b0 + nb])
                    nc.vector.tensor_tensor(
                        X[:, :cn].rearrange("p (b s) -> p b s", s=S),
                        src.rearrange("p (b s) -> p b s", s=S),
                        hp, ALU.add,
                    )
                    nc.scalar.activation(X[:, :cn], X[:, :cn], AF.Tanh)
                    nc.tensor.matmul(
                        pe[:1, :cn], w2T_sb[:, m:m + 1], X[:, :cn],
                        start=(m == 0), stop=(m == KH - 1),
                    )
                nc.scalar.copy(e_sb[0:1, c0:c0 + cn], pe[:1, :cn])
            nc.sync.dma_start(out=d["e"], in_=e_sb[:])
            eN = tiny.tile([P, S], F16, tag="eN")
            nc.sync.dma_start(out=eN[:], in_=d["e"].rearrange("one (b s) -> (one b) s", b=P))

            negm = tiny.tile([P, 1], F32, tag="negm")
            nc.vector.reduce_max(negm[:], eN[:], axis=AX.X, negate=True)
            expe = tiny.tile([P, S], F32, tag="expe")
            sume = tiny.tile([P, 1], F32, tag="sume")
            nc.scalar.activation(expe[:], eN[:], AF.Exp, bias=negm[:], accum_out=sume[:])
            rinv = tiny.tile([P, 1], F32, tag="rinv")
            nc.vector.reciprocal(rinv[:], sume[:])
            nc.vector.tensor_scalar_mul(alpha_sb[:], expe[:], rinv[:])

            # context: ctx[b, :] = sum_s alpha[b, s] * enc[b, s, :]
            ctxA = pp_ctx.tile([P, 512], F32, tag="ctx")
            ctxB = pp_ctx.tile([P, 512], F32, tag="ctx")
            for sb2 in range(S // 2):
                eN2 = encbuf.tile([P, 2 * H], BF16, tag="encb")
                nc.sync.dma_start(out=eN2[:], in_=d["encN"][:, sb2 * 2 * H:(sb2 + 1) * 2 * H])
                for j in range(2):
                    s = sb2 * 2 + j
                    dg = xpool.tile([P, P], BF16, tag="diag")
                    nc.vector.tensor_scalar_mul(dg[:], I_bf[:], alpha_sb[:, s:s + 1])
                    nc.tensor.matmul(ctxA[:], dg[:], eN2[:, j * H: j * H + 512],
                                     start=(s == 0), stop=(s == S - 1))
                    nc.tensor.matmul(ctxB[:], dg[:], eN2[:, j * H + 512:(j + 1) * H],
                                     start=(s == 0), stop=(s == S - 1))
            ctxN = scr.tile([P, H], BF16, tag="ctxN")
            nc.vector.tensor_copy(ctxN[:, :512], ctxA[:])
            nc.vector.tensor_copy(ctxN[:, 512:], ctxB[:])
            for j in range(KH):
                tp = pp_tr.tile([P, P], BF16, tag="tp")
                nc.tensor.transpose(tp[:], ctxN[:, j * P:(j + 1) * P], I_bf[:])
                nc.scalar.copy(ctxT_sb[:, j * P:(j + 1) * P], tp[:])

            # GRU gates
            gi_lhs = [embT_sb[:, (t * 4 + j) * P:(t * 4 + j + 1) * P] for j in range(4)]
            gi_lhs += [ctxT_sb[:, j * P:(j + 1) * P] for j in range(KH)]
            gi_lhs += [ones_sb[:]]

            for c in range(4):          # r,z chunks: 13 wih + 9 whh k-tiles
                wti = wbuf.tile([P, WSLOT], BF16, tag="wbuf")
                nc.sync.dma_start(out=wti[:],
                                  in_=d["wg"][:, c * 22 * 512: c * 22 * 512 + WSLOT])
                wth = wbuf.tile([P, WSLOT], BF16, tag="wbuf")
                nc.sync.dma_start(
                    out=wth[:, :KHH * 512],
                    in_=d["wg"][:, c * 22 * 512 + WSLOT: (c + 1) * 22 * 512])
                ps = pp_mm.tile([P, 512], F32, tag="ps")
                for k in range(KX):
                    nc.tensor.matmul(ps[:], gi_lhs[k], wti[:, k * 512:(k + 1) * 512],
                                     start=(k == 0), stop=False)
                for k in range(KHH):
                    nc.tensor.matmul(ps[:], gh_lhs[k], wth[:, k * 512:(k + 1) * 512],
                                     start=False, stop=(k == KHH - 1))
                # sigmoid(u) = 0.5 + 0.5*tanh(u/2)
                nc.scalar.activation(rz_sb[:, c * 512:(c + 1) * 512], ps[:],
                                     AF.Tanh, scale=0.5)
            nc.vector.tensor_scalar(rz_sb[:], rz_sb[:], 0.5, 0.5, ALU.mult, ALU.add)

            ibase = 4 * 22 * 512
            for c in range(2):          # i_n chunks: 13 k-tiles
                wt = wbuf.tile([P, WSLOT], BF16, tag="wbuf")
                nc.sync.dma_start(
                    out=wt[:],
                    in_=d["wg"][:, ibase + c * KX * 512: ibase + (c + 1) * KX * 512])
                ps = pp_mm.tile([P, 512], F32, tag="ps")
                for k in range(KX):
                    nc.tensor.matmul(ps[:], gi_lhs[k], wt[:, k * 512:(k + 1) * 512],
                                     start=(k == 0), stop=(k == KX - 1))
                nc.vector.tensor_copy(in_sb[:, c * 512:(c + 1) * 512], ps[:])
            hbase = ibase + 2 * KX * 512
            nstate = scr.tile([P, H], F32, tag="nstate")
            for c in range(2):          # h_n chunks: 9 k-tiles
                wt = wbuf.tile([P, WSLOT], BF16, tag="wbuf")
                nc.sync.dma_start(
                    out=wt[:, :KHH * 512],
                    in_=d["wg"][:, hbase + c * KHH * 512: hbase + (c + 1) * KHH * 512])
                ps = pp_mm.tile([P, 512], F32, tag="ps")
                for k in range(KHH):
                    nc.tensor.matmul(ps[:], gh_lhs[k], wt[:, k * 512:(k + 1) * 512],
                                     start=(k == 0), stop=(k == KHH - 1))
                # n = tanh(i_n + r * h_n)
                rh = tiny.tile([P, 512], F32, tag="rh")
                nc.vector.tensor_tensor(rh[:], rz_sb[:, c * 512:(c + 1) * 512], ps[:], ALU.mult)
                pre = tiny.tile([P, 512], F32, tag="pre")
                nc.vector.tensor_tensor(pre[:], rh[:], in_sb[:, c * 512:(c + 1) * 512], ALU.add)
                nc.scalar.activation(nstate[:, c * 512:(c + 1) * 512], pre[:], AF.Tanh)
            # h_new = n + z * (h - n)
            dtile = scr.tile([P, H], F32, tag="dtile")
            nc.vector.tensor_tensor(dtile[:], hN_sb[:], nstate[:], ALU.subtract)
            nc.vector.tensor_tensor(dtile[:], rz_sb[:, H:2 * H], dtile[:], ALU.mult)
            nc.vector.tensor_tensor(hN_sb[:], nstate[:], dtile[:], ALU.add)
            hbf = scr.tile([P, H], BF16, tag="hbf")
            nc.vector.tensor_copy(hbf[:], hN_sb[:])
            for j in range(KH):
                tp = pp_tr.tile([P, P], BF16, tag="tp")
                nc.tensor.transpose(tp[:], hbf[:, j * P:(j + 1) * P], I_bf[:])
                nc.scalar.copy(hT_sb[:, j * P:(j + 1) * P], tp[:])

            # logits with new h
            oT_d = d["o0T"] if t != 1 else d["o1T"]
            h_lhs = [hT_sb[:, j * P:(j + 1) * P] for j in range(KH)] + [ones_sb[:]]
            for c in range(NVC):
                wt = wbuf.tile([P, WSLOT], BF16, tag="wbuf")
                nc.sync.dma_start(
                    out=wt[:, :KHH * 512],
                    in_=oT_d[:, c * KHH * 512:(c + 1) * KHH * 512])
                ps = pp_mm.tile([P, 512], F32, tag="ps")
                for k in range(KHH):
                    nc.tensor.matmul(ps[:], h_lhs[k], wt[:, k * 512:(k + 1) * 512],
                                     start=(k == 0), stop=(k == KHH - 1))
                nc.scalar.copy(lo_sb[:, c * 512:(c + 1) * 512], ps[:])
            negmax, se = stats[t]
            nc.vector.reduce_max(negmax[:], lo_sb[:, :V], axis=AX.X, negate=True)
            junk = scr.tile([P, VP], F16, tag="scratch16")
            nc.scalar.activation(junk[:, :V], lo_sb[:, :V], AF.Exp,
                                 bias=negmax[:], accum_out=se[:])
            nc.sync.dma_start(out=d["lo"][t], in_=lo_sb[:])

        # ---- tail: logp = lo - max - ln(se) ----
        for t in range(MAXLEN):
            negmax, se = stats[t]
            lnse = tiny.tile([P, 1], F32, tag="lnse")
            nc.scalar.activation(lnse[:], se[:], AF.Ln)
            lob = scr.tile([P, VP], F16, tag="scratch16")
            nc.sync.dma_start(out=lob[:], in_=d["lo"][t])
            for half in range(2):
                logp = scr.tile([P, 2000], F32, tag="logp")
                nc.vector.tensor_scalar(
                    logp[:], lob[:, half * 2000:(half + 1) * 2000],
                    negmax[:], lnse[:], ALU.add, ALU.subtract)
                nc.sync.dma_start(out=d["out"][t][:, half * 2000:(half + 1) * 2000],
                                  in_=logp[:])


_NC_CACHE = None


def _get_nc():
    global _NC_CACHE
    if _NC_CACHE is None:
        _NC_CACHE = build_nc()
    return _NC_CACHE


def kernel(**inputs):
    from concourse.bass_utils import run_bass_kernel_spmd

    nc = _get_nc()
    in_maps = [prep_core_inputs(inputs, c) for c in range(NCORES)]
    trace = os.environ.get("KERNEL_TRACE", "0") == "1"
    res = run_bass_kernel_spmd(nc, in_maps, core_ids=list(range(NCORES)), trace=trace)
    kernel.last_results = res
    out = np.concatenate([r["out"] for r in res.results], axis=1)
    return out
